# revision 26
# baseline (speedup 1.0000x reference)
"""Trainium2 Bass kernel for batched attention with key==value.

reference:
    score   = einsum("bqd,bkd->bqk", query, values)        # [B, L, L]
    weights = softmax(score, axis=-1)                      # [B, L, L]
    context = einsum("bqk,bkd->bqd", weights, values)      # [B, L, D]
    return (context, weights)

Sharding: batch dim (B=8) across the 8 NeuronCores, one batch element
per core, no cross-core communication.

Per-core algorithm (Q, V: [L=2048, D=1024] fp32):
  Phase A (no PE work):
    - 512-row blocks of Q/V loaded as [128, 16KB] (4 consecutive rows
      per partition -> 16KB DMA descriptors, ~4x queue bandwidth),
      DVE-cast to fp16, then 4 strided xbar DMA-transposes per block
      into QT/VT [d-part, seq] (fp16).
    - VB [k-part, d] (fp16) rebuilt from VT by 8 more xbar transposes
      (no extra HBM traffic).
  Main loop over 128-row q-tiles (software pipelined):
    S = QT.T @ VT accumulated in PSUM via fp16 matmuls (full PE rate;
        ~11-bit mantissa inputs, plenty for softmax @ 2e-2 gate);
    per-chunk row-max (DVE) -> global negmax -> fused exp(x - max) +
    row-sum on ACT -> E (fp16);
    ONE whole-row-block DMA-transpose of E into PT [k-part, k-tile, q];
    weights out = E * (1/rowsum) on ACT (fp32) -> scalar-queue DMA;
    C = PT.T @ VB accumulated in PSUM (fp16 matmuls);
    context out = C * (1/rowsum) on DVE -> gpsimd DMA.
  Queue discipline: sync = xbar transposes only; scalar = input stage
  loads + weights out; gpsimd = context out.  PE runs only matmuls.
"""

import sys

sys.path.insert(0, "/opt/trn_rl_repo")

import numpy as np

B = 8
L = 2048
D = 1024
P = 128
CH = 512            # S-chunk width (one PSUM bank of fp32)
RPB = 4             # rows per partition in stage loads (16KB descriptors)


def build(L=L, D=D):
    import concourse.bass as bass  # noqa: F401
    import concourse.mybir as mybir
    import concourse.tile as tile
    from concourse.tile import add_dep_helper
    from concourse import bacc
    from contextlib import ExitStack

    f32 = mybir.dt.float32
    f16 = mybir.dt.float16
    X = mybir.AxisListType.X
    Exp = mybir.ActivationFunctionType.Exp
    Copy = mybir.ActivationFunctionType.Copy

    NT = L // P          # 128-row tiles along seq
    ND = D // P          # 128-wide tiles along d
    ch = min(CH, L)
    NCH = L // ch        # S chunks per row-block
    CH2 = min(512, D)
    NDB = D // CH2       # C chunks per row-block
    BR = P * RPB         # rows per stage block (512)
    NB = L // BR         # stage blocks per matrix

    nc = bacc.Bacc()
    q_ext = nc.declare_dram_parameter("query", [L, D], f32, isOutput=False)
    v_ext = nc.declare_dram_parameter("values", [L, D], f32, isOutput=False)
    w_ext = nc.declare_dram_parameter("weights", [L, L], f32, isOutput=True)
    c_ext = nc.declare_dram_parameter("context", [L, D], f32, isOutput=True)

    with tile.TileContext(nc) as tc, ExitStack() as ctx:
        big_pool = ctx.enter_context(tc.tile_pool(name="big", bufs=1))
        stage_pool = ctx.enter_context(tc.tile_pool(name="stage", bufs=3))
        work_pool = ctx.enter_context(tc.tile_pool(name="work", bufs=2))
        pf_pool = ctx.enter_context(tc.tile_pool(name="pf", bufs=2))
        cs_pool = ctx.enter_context(tc.tile_pool(name="cs", bufs=1))
        sa_psum = ctx.enter_context(tc.tile_pool(name="sa_ps", bufs=2, space="PSUM"))
        sb_psum = ctx.enter_context(tc.tile_pool(name="sb_ps", bufs=1, space="PSUM"))
        c_psum = ctx.enter_context(tc.tile_pool(name="c_ps", bufs=2, space="PSUM"))

        VT = big_pool.tile([P, ND, L], f16)     # [d%128, d//128, k (pos-permuted)]
        # C-matmul rhs: the fp16 V stages themselves ARE the k-permuted
        # [k-part, d] layout (VB_perm[a, kt, d] = vstg[kt//RPB][a, kt%RPB, d])
        vstg_pool = ctx.enter_context(tc.tile_pool(name="vstg", bufs=NB))
        vstgs = []
        qt_pool = ctx.enter_context(tc.tile_pool(name="qt", bufs=2))
        qtiles = {}                              # i -> [d%128, d//128, q-in-tile]

        # ---- Phase A (no PE work) ----
        load_insts = []

        def load_stage(src_ext, blk, eng, after=None, persist=False):
            """Load BR=512 rows as [128, RPB*D] (16KB contiguous per
            partition: partition p holds rows blk*BR+RPB*p..+RPB-1),
            cast to fp16.  eng=nc.gpsimd uses SWDGE cast-DMA (f32->f16
            in one step, no f32 stage)."""
            src = src_ext[blk * BR : (blk + 1) * BR, :].rearrange(
                "(p j) d -> p (j d)", p=P
            )
            if persist:
                stg16 = vstg_pool.tile([P, RPB, D], f16, tag="vstg")
            else:
                stg16 = stage_pool.tile([P, RPB, D], f16, tag="stage16")
            flat = stg16[:].rearrange("p j d -> p (j d)")
            if eng is nc.gpsimd:
                ld = eng.dma_start(flat, src)  # cast during DMA
                load_insts.append(ld)
                if after is not None:
                    add_dep_helper(ld.ins, after.ins, sync=True, reason="phaseA order")
                return stg16
            stg = stage_pool.tile([P, RPB * D], f32, tag="stage")
            ld = eng.dma_start(stg[:], src)
            load_insts.append(ld)
            if after is not None:
                add_dep_helper(ld.ins, after.ins, sync=True, reason="phaseA order")
            if blk % 2 == 0:
                nc.vector.tensor_copy(flat, stg[:])
            else:
                nc.scalar.copy(flat, stg[:])
            return stg16

        # DMA-transposes serialize globally against all copy DMAs (xbar
        # mode switching), so phase A batches: loads first (both HWDGE
        # queues in parallel), then transposes back-to-back.
        def q_transpose(stg16, blk, after=None):
            # Q: permuted q order is fine (outputs use strided-row APs).
            # q-tile i=(blk*RPB+j) covers rows {blk*BR + RPB*c + j}.
            # One whole-block xbar transpose: out[a,(j,dt),c] =
            # stg16[c, j, dt*128+a].
            qt4 = qt_pool.tile([P, RPB, ND, P], f16, tag="qt")
            for j in range(RPB):
                qtiles[blk * RPB + j] = qt4[:, j]
            tr = nc.sync.dma_start(qt4[:], stg16[:], transpose=True)
            if after is not None:
                add_dep_helper(tr.ins, after.ins, sync=True, reason="phaseA order")
            return tr

        def v_transpose(stg16, blk):
            # V stays k-PERMUTED: VT column pos = blk*BR + j*128 + c holds
            # k-row blk*BR + RPB*c + j.  VB / E / PT all inherit the same
            # pos-ordering consistently (softmax + sum-over-k are
            # permutation invariant); only the weights output un-permutes
            # (strided-read AP in the pf copy).
            # per-j transpose: VT[:, :, (blk*RPB+j)*128+c] = V[blk*BR+RPB*c+j, :]
            for j in range(RPB):
                pos = blk * RPB + j
                nc.sync.dma_start(
                    VT[:, :, pos * P : (pos + 1) * P],
                    stg16[:, j, :],
                    transpose=True,
                )

        q0_st = load_stage(q_ext, 0, nc.scalar)
        v_engs = [nc.sync, nc.scalar, nc.sync, nc.scalar][:NB]
        v_st = [
            load_stage(v_ext, blk, v_engs[blk % len(v_engs)], persist=True)
            for blk in range(NB)
        ]
        vstgs.extend(v_st)
        # Interleave per-j Q0 transposes with per-block V transposes so
        # S(0).c0 unblocks after only 2 small transposes.  First transpose
        # gated on the first 3 loads (the global copy<->transpose
        # serialization otherwise ping-pongs loads and transposes).
        qt4_0 = qt_pool.tile([P, RPB, ND, P], f16, tag="qt")
        for j in range(RPB):
            qtiles[j] = qt4_0[:, j]
        first_tr = None
        for j in range(min(RPB, NB)):
            tr = nc.sync.dma_start(qt4_0[:, j], q0_st[:, j, :], transpose=True)
            if first_tr is None:
                first_tr = tr
                for ld in load_insts[:3]:
                    add_dep_helper(tr.ins, ld.ins, sync=True, reason="phaseA loads first")
            v_transpose(v_st[j], j)
        for j in range(min(RPB, NB), RPB):
            nc.sync.dma_start(qt4_0[:, j], q0_st[:, j, :], transpose=True)
        for blk in range(min(RPB, NB), NB):
            v_transpose(v_st[blk], blk)
        for blk in range(1, NB):
            stg16 = load_stage(q_ext, blk, nc.scalar)
            q_transpose(stg16, blk)

        # ---- Main loop ----
        def emit_C(cargs):
            ptb, rcp, i = cargs
            cs = cs_pool.tile([P, D], f32, tag="cs")
            for db in range(NDB):
                sl = slice(db * CH2, (db + 1) * CH2)
                c_ps = c_psum.tile([P, CH2], f32, tag="c_ps")
                for kt in range(NT):
                    nc.tensor.matmul(
                        c_ps[:],
                        ptb[:, kt, :],
                        vstgs[kt // RPB][:, kt % RPB, sl],
                        start=(kt == 0),
                        stop=(kt == NT - 1),
                    )
                nc.vector.tensor_scalar_mul(cs[:, sl], c_ps[:], rcp[:])
            blk, j = divmod(i, RPB)
            c_rows = c_ext[blk * BR : (blk + 1) * BR, :].rearrange(
                "(c j) d -> j c d", j=RPB
            )
            nc.gpsimd.dma_start(c_rows[j], cs[:])

        deferred = [None]
        for i in range(NT):
            halfA = (NCH + 1) // 2
            sA = sa_psum.tile([P, halfA * ch], f32, tag="sa_ps")
            sB = None
            if NCH > halfA:
                sB = sb_psum.tile([P, (NCH - halfA) * ch], f32, tag="sb_ps")

            def s_chunk(nb):
                if nb < halfA:
                    return sA[:, nb * ch : (nb + 1) * ch]
                return sB[:, (nb - halfA) * ch : (nb - halfA + 1) * ch]

            mpart = work_pool.tile([P, NCH], f32, tag="mpart")
            for nb in range(NCH):
                sl = slice(nb * ch, (nb + 1) * ch)
                sc = s_chunk(nb)
                for dt in range(ND):
                    nc.tensor.matmul(
                        sc,
                        qtiles[i][:, dt, :],
                        VT[:, dt, sl],
                        start=(dt == 0),
                        stop=(dt == ND - 1),
                    )
                nc.vector.reduce_max(mpart[:, nb : nb + 1], sc, axis=X)

            negmax = work_pool.tile([P, 1], f32, tag="negmax")
            nc.vector.reduce_max(negmax[:], mpart[:], axis=X, negate=True)

            eb = work_pool.tile([P, L], f16, tag="eb")
            srows = work_pool.tile([P, NCH], f32, tag="srows")
            for nb in range(NCH):
                sl = slice(nb * ch, (nb + 1) * ch)
                nc.scalar.activation(
                    eb[:, sl],
                    s_chunk(nb),
                    Exp,
                    bias=negmax[:],
                    scale=1.0,
                    accum_out=srows[:, nb : nb + 1],
                )

            rsum = work_pool.tile([P, 1], f32, tag="rsum")
            nc.vector.reduce_sum(rsum[:], srows[:], axis=X)
            rcp = work_pool.tile([P, 1], f32, tag="rcp")
            nc.vector.reciprocal(rcp[:], rsum[:])

            # transposed exp(S) in two k-halves so C(i)'s first matmuls
            # unblock after half 1: ptb[k%128, k//128, q] = eb[q, k]
            ptb = work_pool.tile([P, NT, P], f16, tag="ptb")
            hk = NT // 2
            nc.sync.dma_start(
                ptb[:, :hk, :], eb[:, : hk * P], transpose=True
            )
            nc.sync.dma_start(
                ptb[:, hk:, :], eb[:, hk * P :], transpose=True
            )

            # normalized weights output (ACT: copy with per-partition scale)
            pf = pf_pool.tile([P, L], f32, tag="pf")
            # un-permute k: true col b*BR + RPB*c + j reads pos b*BR + j*128 + c
            eb_perm = eb[:].rearrange("p (b j c) -> p b c j", b=NB, j=RPB, c=P)
            pf_4d = pf[:].rearrange("p (b c j) -> p b c j", b=NB, c=P, j=RPB)
            nc.vector.tensor_scalar_mul(pf_4d, eb_perm, rcp[:])
            blk, j = divmod(i, RPB)
            w_rows = w_ext[blk * BR : (blk + 1) * BR, :].rearrange(
                "(c j) k -> j c k", j=RPB
            )
            nc.scalar.dma_start(w_rows[j], pf[:])

            if deferred[0] is not None:
                emit_C(deferred[0])
            deferred[0] = (ptb, rcp, i)

        emit_C(deferred[0])

    nc.compile()
    return nc


_NC = None


def kernel(query, values):
    from concourse.bass_utils import run_bass_kernel_spmd

    global _NC
    if _NC is None:
        _NC = build()

    q = np.ascontiguousarray(np.asarray(query, dtype=np.float32))
    v = np.ascontiguousarray(np.asarray(values, dtype=np.float32))
    assert q.shape == (B, L, D) and v.shape == (B, L, D), (q.shape, v.shape)

    in_maps = [{"query": q[i], "values": v[i]} for i in range(B)]
    res = run_bass_kernel_spmd(_NC, in_maps, core_ids=list(range(B)))
    context = np.stack([res.results[i]["context"] for i in range(B)])
    weights = np.stack([res.results[i]["weights"] for i in range(B)])
    return context, weights


# revision 27
# speedup vs baseline: 1.1749x; 1.1749x over previous
"""Trainium2 Bass kernel for batched attention with key==value.

reference:
    score   = einsum("bqd,bkd->bqk", query, values)        # [B, L, L]
    weights = softmax(score, axis=-1)                      # [B, L, L]
    context = einsum("bqk,bkd->bqd", weights, values)      # [B, L, D]
    return (context, weights)

Sharding: batch dim (B=8) across the 8 NeuronCores, one batch element
per core, no cross-core communication.

Per-core algorithm (Q, V: [L=2048, D=1024] fp32):
  Phase A (no PE work):
    - 512-row blocks of Q/V loaded as [128, 16KB] (4 consecutive rows
      per partition -> 16KB DMA descriptors, ~4x queue bandwidth),
      DVE-cast to fp16, then 4 strided xbar DMA-transposes per block
      into QT/VT [d-part, seq] (fp16).
    - VB [k-part, d] (fp16) rebuilt from VT by 8 more xbar transposes
      (no extra HBM traffic).
  Main loop over 128-row q-tiles (software pipelined):
    S = QT.T @ VT accumulated in PSUM via fp16 matmuls (full PE rate;
        ~11-bit mantissa inputs, plenty for softmax @ 2e-2 gate);
    per-chunk row-max (DVE) -> global negmax -> fused exp(x - max) +
    row-sum on ACT -> E (fp16);
    ONE whole-row-block DMA-transpose of E into PT [k-part, k-tile, q];
    weights out = E * (1/rowsum) on ACT (fp32) -> scalar-queue DMA;
    C = PT.T @ VB accumulated in PSUM (fp16 matmuls);
    context out = C * (1/rowsum) on DVE -> gpsimd DMA.
  Queue discipline: sync = xbar transposes only; scalar = input stage
  loads + weights out; gpsimd = context out.  PE runs only matmuls.
"""

import sys

sys.path.insert(0, "/opt/trn_rl_repo")

import numpy as np

B = 8
L = 2048
D = 1024
P = 128
CH = 512            # S-chunk width (one PSUM bank of fp32)
RPB = 4             # rows per partition in stage loads (16KB descriptors)


def build(L=L, D=D):
    import concourse.bass as bass  # noqa: F401
    import concourse.mybir as mybir
    import concourse.tile as tile
    from concourse.tile import add_dep_helper
    from concourse import bacc
    from contextlib import ExitStack

    f32 = mybir.dt.float32
    f16 = mybir.dt.float16
    X = mybir.AxisListType.X
    Exp = mybir.ActivationFunctionType.Exp
    Copy = mybir.ActivationFunctionType.Copy

    NT = L // P          # 128-row tiles along seq
    ND = D // P          # 128-wide tiles along d
    ch = min(CH, L)
    NCH = L // ch        # S chunks per row-block
    CH2 = min(512, D)
    NDB = D // CH2       # C chunks per row-block
    BR = P * RPB         # rows per stage block (512)
    NB = L // BR         # stage blocks per matrix

    nc = bacc.Bacc()
    q_ext = nc.declare_dram_parameter("query", [L, D], f32, isOutput=False)
    v_ext = nc.declare_dram_parameter("values", [L, D], f32, isOutput=False)
    w_ext = nc.declare_dram_parameter("weights", [L, L], f32, isOutput=True)
    c_ext = nc.declare_dram_parameter("context", [L, D], f32, isOutput=True)

    with tile.TileContext(nc) as tc, ExitStack() as ctx:
        big_pool = ctx.enter_context(tc.tile_pool(name="big", bufs=1))
        stage_pool = ctx.enter_context(tc.tile_pool(name="stage", bufs=3))
        work_pool = ctx.enter_context(tc.tile_pool(name="work", bufs=2))
        pf_pool = ctx.enter_context(tc.tile_pool(name="pf", bufs=2))
        cs_pool = ctx.enter_context(tc.tile_pool(name="cs", bufs=1))
        sa_psum = ctx.enter_context(tc.tile_pool(name="sa_ps", bufs=2, space="PSUM"))
        sb_psum = ctx.enter_context(tc.tile_pool(name="sb_ps", bufs=1, space="PSUM"))
        c_psum = ctx.enter_context(tc.tile_pool(name="c_ps", bufs=2, space="PSUM"))

        VT = big_pool.tile([P, ND, L], f16)     # [d%128, d//128, k (pos-permuted)]
        # C-matmul rhs: the fp16 V stages themselves ARE the k-permuted
        # [k-part, d] layout (VB_perm[a, kt, d] = vstg[kt//RPB][a, kt%RPB, d])
        vstg_pool = ctx.enter_context(tc.tile_pool(name="vstg", bufs=NB))
        vstgs = []
        qt_pool = ctx.enter_context(tc.tile_pool(name="qt", bufs=2))
        qtiles = {}                              # i -> [d%128, d//128, q-in-tile]

        # ---- Phase A (no PE work) ----
        load_insts = []

        def load_stage(src_ext, blk, eng, after=None, persist=False):
            """Load BR=512 rows as [128, RPB*D] (16KB contiguous per
            partition: partition p holds rows blk*BR+RPB*p..+RPB-1),
            cast to fp16.  eng=nc.gpsimd uses SWDGE cast-DMA (f32->f16
            in one step, no f32 stage)."""
            src = src_ext[blk * BR : (blk + 1) * BR, :].rearrange(
                "(p j) d -> p (j d)", p=P
            )
            if persist:
                stg16 = vstg_pool.tile([P, RPB, D], f16, tag="vstg")
            else:
                stg16 = stage_pool.tile([P, RPB, D], f16, tag="stage16")
            flat = stg16[:].rearrange("p j d -> p (j d)")
            if eng is nc.gpsimd:
                ld = eng.dma_start(flat, src)  # cast during DMA
                load_insts.append(ld)
                if after is not None:
                    add_dep_helper(ld.ins, after.ins, sync=True, reason="phaseA order")
                return stg16
            stg = stage_pool.tile([P, RPB * D], f32, tag="stage")
            ld = eng.dma_start(stg[:], src)
            load_insts.append(ld)
            if after is not None:
                add_dep_helper(ld.ins, after.ins, sync=True, reason="phaseA order")
            if blk % 2 == 0:
                nc.vector.tensor_copy(flat, stg[:])
            else:
                nc.scalar.copy(flat, stg[:])
            return stg16

        # DMA-transposes serialize globally against all copy DMAs (xbar
        # mode switching), so phase A batches: loads first (both HWDGE
        # queues in parallel), then transposes back-to-back.
        def q_transpose(stg16, blk, after=None):
            # Q: permuted q order is fine (outputs use strided-row APs).
            # q-tile i=(blk*RPB+j) covers rows {blk*BR + RPB*c + j}.
            # One whole-block xbar transpose: out[a,(j,dt),c] =
            # stg16[c, j, dt*128+a].
            qt4 = qt_pool.tile([P, RPB, ND, P], f16, tag="qt")
            for j in range(RPB):
                qtiles[blk * RPB + j] = qt4[:, j]
            tr = nc.sync.dma_start(qt4[:], stg16[:], transpose=True)
            if after is not None:
                add_dep_helper(tr.ins, after.ins, sync=True, reason="phaseA order")
            return tr

        def v_transpose(stg16, blk):
            # V stays k-PERMUTED: VT column pos = blk*BR + j*128 + c holds
            # k-row blk*BR + RPB*c + j.  VB / E / PT all inherit the same
            # pos-ordering consistently (softmax + sum-over-k are
            # permutation invariant); only the weights output un-permutes
            # (strided-read AP in the pf copy).
            # per-j transpose: VT[:, :, (blk*RPB+j)*128+c] = V[blk*BR+RPB*c+j, :]
            for j in range(RPB):
                pos = blk * RPB + j
                nc.sync.dma_start(
                    VT[:, :, pos * P : (pos + 1) * P],
                    stg16[:, j, :],
                    transpose=True,
                )

        q0_st = load_stage(q_ext, 0, nc.scalar)
        v_engs = [nc.sync, nc.scalar, nc.sync, nc.scalar][:NB]
        v_st = [
            load_stage(v_ext, blk, v_engs[blk % len(v_engs)], persist=True)
            for blk in range(NB)
        ]
        vstgs.extend(v_st)
        # Interleave per-j Q0 transposes with per-block V transposes so
        # S(0).c0 unblocks after only 2 small transposes.  First transpose
        # gated on the first 3 loads (the global copy<->transpose
        # serialization otherwise ping-pongs loads and transposes).
        qt4_0 = qt_pool.tile([P, RPB, ND, P], f16, tag="qt")
        for j in range(RPB):
            qtiles[j] = qt4_0[:, j]
        first_tr = None
        for j in range(min(RPB, NB)):
            tr = nc.sync.dma_start(qt4_0[:, j], q0_st[:, j, :], transpose=True)
            if first_tr is None:
                first_tr = tr
                for ld in load_insts[:3]:
                    add_dep_helper(tr.ins, ld.ins, sync=True, reason="phaseA loads first")
            v_transpose(v_st[j], j)
        for j in range(min(RPB, NB), RPB):
            nc.sync.dma_start(qt4_0[:, j], q0_st[:, j, :], transpose=True)
        for blk in range(min(RPB, NB), NB):
            v_transpose(v_st[blk], blk)
        for blk in range(1, NB):
            stg16 = load_stage(q_ext, blk, nc.scalar)
            q_transpose(stg16, blk)

        # ---- Main loop ----
        def emit_C(cargs):
            ptb, rcp, i = cargs
            cs = cs_pool.tile([P, D], f32, tag="cs")
            for db in range(NDB):
                sl = slice(db * CH2, (db + 1) * CH2)
                c_ps = c_psum.tile([P, CH2], f32, tag="c_ps")
                for kt in range(NT):
                    nc.tensor.matmul(
                        c_ps[:],
                        ptb[:, kt, :],
                        vstgs[kt // RPB][:, kt % RPB, sl],
                        start=(kt == 0),
                        stop=(kt == NT - 1),
                    )
                nc.vector.tensor_scalar_mul(cs[:, sl], c_ps[:], rcp[:])
            blk, j = divmod(i, RPB)
            c_rows = c_ext[blk * BR : (blk + 1) * BR, :].rearrange(
                "(c j) d -> j c d", j=RPB
            )
            nc.gpsimd.dma_start(c_rows[j], cs[:])

        deferred = [None]
        for i in range(NT):
            halfA = (NCH + 1) // 2
            sA = sa_psum.tile([P, halfA * ch], f32, tag="sa_ps")
            sB = None
            if NCH > halfA:
                sB = sb_psum.tile([P, (NCH - halfA) * ch], f32, tag="sb_ps")

            def s_chunk(nb):
                if nb < halfA:
                    return sA[:, nb * ch : (nb + 1) * ch]
                return sB[:, (nb - halfA) * ch : (nb - halfA + 1) * ch]

            mpart = work_pool.tile([P, NCH], f32, tag="mpart")
            for nb in range(NCH):
                sl = slice(nb * ch, (nb + 1) * ch)
                sc = s_chunk(nb)
                for dt in range(ND):
                    nc.tensor.matmul(
                        sc,
                        qtiles[i][:, dt, :],
                        VT[:, dt, sl],
                        start=(dt == 0),
                        stop=(dt == ND - 1),
                    )
                nc.vector.reduce_max(mpart[:, nb : nb + 1], sc, axis=X)

            negmax = work_pool.tile([P, 1], f32, tag="negmax")
            nc.vector.reduce_max(negmax[:], mpart[:], axis=X, negate=True)

            eb = work_pool.tile([P, L], f16, tag="eb")
            srows = work_pool.tile([P, NCH], f32, tag="srows")
            for nb in range(NCH):
                sl = slice(nb * ch, (nb + 1) * ch)
                nc.scalar.activation(
                    eb[:, sl],
                    s_chunk(nb),
                    Exp,
                    bias=negmax[:],
                    scale=1.0,
                    accum_out=srows[:, nb : nb + 1],
                )

            rsum = work_pool.tile([P, 1], f32, tag="rsum")
            nc.vector.reduce_sum(rsum[:], srows[:], axis=X)
            rcp = work_pool.tile([P, 1], f32, tag="rcp")
            nc.vector.reciprocal(rcp[:], rsum[:])

            # whole-row-block transposed exp(S): ptb[k%128, k//128, q] = eb[q, k]
            ptb = work_pool.tile([P, NT, P], f16, tag="ptb")
            nc.sync.dma_start(ptb[:], eb[:], transpose=True)

            # normalized weights output (ACT: copy with per-partition scale)
            pf = pf_pool.tile([P, L], f32, tag="pf")
            # un-permute k: true col b*BR + RPB*c + j reads pos b*BR + j*128 + c
            eb_perm = eb[:].rearrange("p (b j c) -> p b c j", b=NB, j=RPB, c=P)
            pf_4d = pf[:].rearrange("p (b c j) -> p b c j", b=NB, c=P, j=RPB)
            nc.vector.tensor_scalar_mul(pf_4d, eb_perm, rcp[:])
            blk, j = divmod(i, RPB)
            w_rows = w_ext[blk * BR : (blk + 1) * BR, :].rearrange(
                "(c j) k -> j c k", j=RPB
            )
            nc.scalar.dma_start(w_rows[j], pf[:])

            if deferred[0] is not None:
                emit_C(deferred[0])
            deferred[0] = (ptb, rcp, i)

        emit_C(deferred[0])

    nc.compile()
    return nc


_NC = None


def kernel(query, values):
    from concourse.bass_utils import run_bass_kernel_spmd

    global _NC
    if _NC is None:
        _NC = build()

    q = np.ascontiguousarray(np.asarray(query, dtype=np.float32))
    v = np.ascontiguousarray(np.asarray(values, dtype=np.float32))
    assert q.shape == (B, L, D) and v.shape == (B, L, D), (q.shape, v.shape)

    in_maps = [{"query": q[i], "values": v[i]} for i in range(B)]
    res = run_bass_kernel_spmd(_NC, in_maps, core_ids=list(range(B)))
    context = np.stack([res.results[i]["context"] for i in range(B)])
    weights = np.stack([res.results[i]["weights"] for i in range(B)])
    return context, weights
